# revision 12
# baseline (speedup 1.0000x reference)
"""Trainium2 Bass kernel for nn_AndAttention.

B=16384 rows; per row: 2-token self-attention over (x1,x2) [D=1024 each],
concat -> h [2048], then 4x (Linear(2048,2048)+ReLU) and Linear(2048,1024).

Sharding: data-parallel over batch across 8 NeuronCores (2048 rows/core),
weights replicated. No collectives.

Layout strategy (per core):
  - Activations live feature-major in SBUF: hT[feature partition, batch free].
  - 2-token softmax == sigmoid of logit differences; dot products via ACT
    Square-with-accumulate and DVE scalar_tensor_tensor-with-accumulate.
  - Attention combine+transpose fused on the PE:
      psum[d, 0:256] = x1c.T @ [diag(a00)|diag(a10)] + x2c.T @ [diag(a01)|diag(a11)]
    which yields y0^T and y1^T directly (feature-major h0).
  - x is pre-cast to bf16 and interleaved [row, token, 1024] on the host,
    so one gpsimd-ring DMA per 128-row tile fetches both tokens and the
    PE can start layer 1 by ~17us. The scalar queue carries no DMA issues
    during attention (each DMA_DIRECT2D costs ~0.65us of engine time and
    ring-window waits would block ACT compute behind it).
  - Layer 1 interleaves with attention per batch-quarter. Its psum
    evictions run on the GPSIMD engine (idle once x is loaded) so the DVE
    and ACT queues carry only attention work and the next quarter's diag
    coefficients are always ready before the PE finishes the current block.
    Emission order is [attn quarter 0][L1 block 0][attn quarter 1][L1 block
    1]... so the in-order PE queue never waits mid-block.
  - MLP layers: lhsT = pre-transposed bf16 weight tiles streamed on the
    sync ring (4 k-subtiles per DMA, 24-deep pool to prefetch through
    layer 1's 4x weight re-read), rhs = hT; psum evicted with fused
    ReLU+bias on the scalar engine.
  - Last-layer weights + bias stream on the gpsimd ring during layer 2
    (recycling the x-tile SBUF slots), keeping the sync ring clear.
  - Last layer swaps matmul args (lhsT = hT chunk, rhs = W_last^T tiles) so
    psum comes out in natural [batch, out] layout; bias added on DVE from a
    host-replicated bias tile; DMA straight to the output. The final batch
    chunk evicts in 256-col pieces so the tail DMA starts earlier.
"""

import sys

if "/opt/trn_rl_repo" not in sys.path:
    sys.path.insert(0, "/opt/trn_rl_repo")

import numpy as np
import ml_dtypes

import concourse.bass as bass
import concourse.tile as tile
from concourse import bacc, mybir
from concourse.bass_utils import run_bass_kernel_spmd
from concourse.masks import make_identity

P = 128
D = 1024
D2 = 2048
DOUT = 1024
N_LAYERS = 4
N_CORES = 8
B = 16384
BC = B // N_CORES           # rows per core = 2048
BP = BC                     # single pass over the whole core batch
NB_TILES = BC // P          # 16 b-tiles of 128 rows per core
KT = D2 // P                # 16 k tiles (contraction)
MT = D2 // P                # 16 m tiles (layer out features)
KG = 4                      # k-subtiles per weight DMA
NCHUNK = 512                # matmul moving free dim
NQ = 512                    # last-layer o-half width
QT = DOUT // NQ             # 2

f32 = mybir.dt.float32
bf16 = mybir.dt.bfloat16
NP_BF16 = np.dtype(ml_dtypes.bfloat16)
AF = mybir.ActivationFunctionType
ALU = mybir.AluOpType


def build_graph(debug_stage=None):
    nc = bacc.Bacc("TRN2", target_bir_lowering=False, debug=False,
                   num_devices=N_CORES)

    # x1/x2 interleaved host-side: one DMA per tile fetches both tokens
    xz_ext = nc.declare_dram_parameter("xz", [BC, 2, D], bf16, isOutput=False)
    # weight tiles: [l, m, kg, i(128), kk(4), o(128)] bf16 with
    #   wt[l, m, kg, i, kk, o] = Ws[l, m*128+o, (kg*4+kk)*128+i]
    wt_ext = nc.declare_dram_parameter("wt", [N_LAYERS, MT, KT // KG, P, KG, P],
                                       bf16, isOutput=False)
    # last-layer tiles: [j, i(128), r(2), o(1024)] bf16 with
    #   wlt[j,i,r,o] = W_last[o, (2j+r)*128+i]  (k pairs packed per tile)
    wlt_ext = nc.declare_dram_parameter("wlt", [KT // 2, P, 2, DOUT], bf16,
                                        isOutput=False)
    # biases: bst[l, p, m] = bs[l, m*128+p]
    bst_ext = nc.declare_dram_parameter("bst", [N_LAYERS, P, MT], f32,
                                        isOutput=False)
    # b_last replicated across partitions: [128, 1024] bf16
    blb_ext = nc.declare_dram_parameter("blb", [P, DOUT], bf16, isOutput=False)
    out_ext = nc.declare_dram_parameter("out", [BC, DOUT], f32, isOutput=True)
    dbg_ext = None
    if debug_stage is not None:
        dbg_ext = nc.declare_dram_parameter("dbg", [P, KT, BP], bf16,
                                            isOutput=True)

    with tile.TileContext(nc) as tc:
        _trace(nc, tc, xz_ext, wt_ext, wlt_ext, bst_ext, blb_ext,
               out_ext, debug_stage, dbg_ext)
    nc.compile()
    return nc


def _trace(nc, tc, xz_ext, wt_ext, wlt_ext, bst_ext, blb_ext, out_ext,
           debug_stage=None, dbg_ext=None):
    from contextlib import ExitStack
    ctx = ExitStack()
    with ctx:
        const = ctx.enter_context(tc.tile_pool(name="const", bufs=1))
        acts = ctx.enter_context(tc.tile_pool(name="acts", bufs=2))
        wpool = ctx.enter_context(tc.tile_pool(name="wpool", bufs=24))
        # 2KB-per-partition slots: rotates x tiles during attention, then the
        # 16 last-layer weight tiles land in the same slots (disjoint in time)
        xwpool = ctx.enter_context(tc.tile_pool(name="xwpool", bufs=8))
        spool = ctx.enter_context(tc.tile_pool(name="spool", bufs=2))
        stpool = ctx.enter_context(tc.tile_pool(name="stpool", bufs=4))
        smpool = ctx.enter_context(tc.tile_pool(name="smpool", bufs=4))
        dpool = ctx.enter_context(tc.tile_pool(name="dpool", bufs=3))
        mpsum = ctx.enter_context(tc.tile_pool(name="mpsum", bufs=8,
                                               space="PSUM"))

        # constants (tiles declared here; init ops emitted after the first
        # attention DMAs so the gpsimd queue issues xz tiles immediately)
        ident = const.tile([P, P], f32)
        warm = const.tile([P, 1], f32)
        bst_sb = const.tile([P, N_LAYERS * MT], f32)
        blb_sb = const.tile([P, DOUT], bf16)

        def init_consts():
            nc.vector.memset(warm[:], 0.0)
            nc.scalar.activation(warm[:], warm[:], AF.Sigmoid)
            make_identity(nc, ident)
            for l in range(N_LAYERS):
                nc.sync.dma_start(bst_sb[:, l * MT:(l + 1) * MT],
                                  bst_ext.ap()[l])

        # ---------- attention: build h0T [2048 feat, 2048 batch] ----------
        h0 = acts.tile([P, KT, BP], bf16, name="hbuf")
        xc_tiles = {}
        diag_tiles = {}

        def attn_dma(t_lo, t_hi, split=False):
            for t in range(t_lo, t_hi):
                # one DMA on the gpsimd ring per tile; the scalar queue
                # stays pure-compute and the sync ring pure-weights.
                # For the head quarter, split by token across the scalar and
                # gpsimd rings so tile 0's stats can start ~10us.
                xc = xwpool.tile([P, 2, D], bf16, name="xc")
                ap = xz_ext.ap()[t * P:(t + 1) * P, :, :]
                if split:
                    nc.scalar.dma_start(xc[:, 0, :], ap[:, 0, :])
                    nc.gpsimd.dma_start(xc[:, 1, :], ap[:, 1, :])
                else:
                    nc.gpsimd.dma_start(xc[:], ap)
                xc_tiles[t] = (xc[:, 0, :], xc[:, 1, :])

        def attn_stats(t):
            xc1, xc2 = xc_tiles[t]
            stat = smpool.tile([P, 4], f32, name="stat")
            # logits (already include the 1/32 temperature):
            # s11/s22 via ACT Square(x/sqrt(32)) with accumulate,
            # s12 via DVE (x1*(1/32))*x2 with accumulate
            scr = spool.tile([P, D], bf16, name="scr")
            nc.scalar.activation(scr[:], xc1[:], AF.Square,
                                 scale=float(1.0 / np.sqrt(32.0)),
                                 accum_out=stat[:, 0:1])
            scr2 = spool.tile([P, D], bf16, name="scr")
            nc.vector.scalar_tensor_tensor(scr2[:], xc1[:], 1.0 / 32.0,
                                           xc2[:], ALU.mult, ALU.mult,
                                           accum_out=stat[:, 1:2])
            scr3 = spool.tile([P, D], bf16, name="scr")
            nc.scalar.activation(scr3[:], xc2[:], AF.Square,
                                 scale=float(1.0 / np.sqrt(32.0)),
                                 accum_out=stat[:, 2:3])

            # one batched sigmoid over [d0, -d0, d1, -d1] yields
            # [a00, a01, a10, a11] in a single ACT op
            dt_ = smpool.tile([P, 4], f32, name="dt")
            nc.vector.tensor_sub(dt_[:, 0:1], stat[:, 0:1], stat[:, 1:2])
            nc.vector.tensor_sub(dt_[:, 1:2], stat[:, 1:2], stat[:, 0:1])
            nc.vector.tensor_sub(dt_[:, 2:3], stat[:, 1:2], stat[:, 2:3])
            nc.vector.tensor_sub(dt_[:, 3:4], stat[:, 2:3], stat[:, 1:2])
            coef = smpool.tile([P, 4], f32, name="coef")
            nc.scalar.activation(coef[:], dt_[:], AF.Sigmoid)

            # diagA = [diag(a00)|diag(a10)], diagB = [diag(a01)|diag(a11)]
            diagA = dpool.tile([P, 2 * P], bf16, name="diagA")
            nc.vector.tensor_scalar_mul(diagA[:, 0:P], ident[:],
                                        coef[:, 0:1])
            nc.vector.tensor_scalar_mul(diagA[:, P:2 * P], ident[:],
                                        coef[:, 2:3])
            diagB = dpool.tile([P, 2 * P], bf16, name="diagB")
            nc.vector.tensor_scalar_mul(diagB[:, 0:P], ident[:],
                                        coef[:, 1:2])
            nc.vector.tensor_scalar_mul(diagB[:, P:2 * P], ident[:],
                                        coef[:, 3:4])
            diag_tiles[t] = (diagA, diagB)

        def attn_combine(t):
            xc1, xc2 = xc_tiles.pop(t)  # noqa: kept until here for pool rotation
            diagA, diagB = diag_tiles.pop(t)
            col = t * P
            for dc in range(D // P):  # 8 feature chunks
                ps = mpsum.tile([P, NCHUNK], f32, name="mps")
                nc.tensor.matmul(ps[:, 0:2 * P],
                                 xc1[:, dc * P:(dc + 1) * P],
                                 diagA[:], start=True, stop=False)
                nc.tensor.matmul(ps[:, 0:2 * P],
                                 xc2[:, dc * P:(dc + 1) * P],
                                 diagB[:], start=False, stop=True)
                # one strided copy covers both tokens' chunks (k-slices dc
                # and dc+8); DVE strided copies run ~1.5x faster than ACT's,
                # so DVE takes 5 of the 8 chunks
                dst = h0[:, dc::8, col:col + P]
                if dc in (0, 3, 6):
                    nc.scalar.copy(dst, ps[:, 0:2 * P])
                else:
                    nc.vector.tensor_copy(dst, ps[:, 0:2 * P])

        def layer1_block(h_in, h_out, n, hooks=None, preloaded=None):
            for m in range(MT):
                ps = mpsum.tile([P, NCHUNK], f32, name="mps")
                for kg in range(KT // KG):
                    if preloaded is not None and (m, kg) in preloaded:
                        wt = preloaded.pop((m, kg))
                    else:
                        wt = wpool.tile([P, KG, P], bf16, name="wt")
                        nc.sync.dma_start(wt[:], wt_ext.ap()[0, m, kg])
                    for kk in range(KG):
                        k = kg * KG + kk
                        nc.tensor.matmul(
                            ps[:], wt[:, kk, :],
                            h_in[:, k, n * NCHUNK:(n + 1) * NCHUNK],
                            start=(k == 0), stop=(k == KT - 1))
                # evictions alternate ACT/DVE so neither queue blocks the
                # interleaved attention work of the next quarter for long
                dst = h_out[:, m, n * NCHUNK:(n + 1) * NCHUNK]
                if m % 2 == 0:
                    nc.scalar.activation(dst, ps[:], AF.Relu,
                                         bias=bst_sb[:, m:m + 1])
                else:
                    nc.vector.tensor_scalar(dst, ps[:], bst_sb[:, m:m + 1],
                                            0.0, ALU.add, ALU.max)
                if hooks and m in hooks:
                    hooks[m]()

        if debug_stage == "attn":
            attn_dma(0, NB_TILES)
            init_consts()
            for t in range(NB_TILES):
                attn_stats(t)
                attn_combine(t)
            nc.sync.dma_start(dbg_ext.ap()[:, :, :], h0[:])
            return

        # interleave: each layer-1 n-chunk only needs a quarter of the batch
        # columns; the next quarter's attention is hooked into the m-loop
        # (stats at m=1,5,9,13; PE combines at m=3,7,11,15) so its diag
        # coefficients always lead the PE and no in-order queue blocks.
        # The first two m-tiles' weights are issued before the rest so
        # their data beats h0 readiness on the slow-ramping sync ring.
        attn_dma(0, 4)
        init_consts()
        preloaded = {}
        for m in range(2):
            for kg in range(KT // KG):
                wt = wpool.tile([P, KG, P], bf16, name="wt")
                nc.sync.dma_start(wt[:], wt_ext.ap()[0, m, kg])
                preloaded[(m, kg)] = wt
        for t in range(4):
            attn_stats(t)
            attn_combine(t)
        h1 = acts.tile([P, KT, BP], bf16, name="hbuf")
        for n in range(4):
            hooks = None
            if n < 3:
                attn_dma(4 * (n + 1), 4 * (n + 2))
                base = 4 * (n + 1)
                hooks = {}
                for i in range(4):
                    hooks[4 * i + 1] = (lambda t=base + i: attn_stats(t))
                    hooks[4 * i + 3] = (lambda t=base + i: attn_combine(t))
            layer1_block(h0, h1, n, hooks,
                         preloaded=preloaded if n == 0 else None)
        h = h1

        # ---------- MLP layers 2..4 (feature-major) ----------
        wl_tiles = []
        for l in range(1, N_LAYERS):
            if l == 2:
                # last-layer weights + bias stream on the (now idle) gpsimd
                # ring into the recycled x-tile slots
                nc.gpsimd.dma_start(blb_sb[:], blb_ext.ap()[:, :])
                for j in range(KT // 2):
                    wl = xwpool.tile([P, 2, DOUT], bf16, name="xc")
                    nc.gpsimd.dma_start(wl[:], wlt_ext.ap()[j])
                    wl_tiles.append(wl)
            hout = acts.tile([P, KT, BP], bf16, name="hbuf")
            for m in range(MT):
                pss = [mpsum.tile([P, NCHUNK], f32, name="mps")
                       for _ in range(BP // NCHUNK)]
                for kg in range(KT // KG):
                    wt = wpool.tile([P, KG, P], bf16, name="wt")
                    nc.sync.dma_start(wt[:], wt_ext.ap()[l, m, kg])
                    for kk in range(KG):
                        k = kg * KG + kk
                        first = (k == 0)
                        last = (k == KT - 1)
                        for nn in range(BP // NCHUNK):
                            nc.tensor.matmul(
                                pss[nn][:], wt[:, kk, :],
                                h[:, k, nn * NCHUNK:(nn + 1) * NCHUNK],
                                start=first, stop=last)
                bias = bst_sb[:, l * MT + m:l * MT + m + 1]
                for nn in range(BP // NCHUNK):
                    nc.scalar.activation(hout[:, m, nn * NCHUNK:(nn + 1) * NCHUNK],
                                         pss[nn][:], AF.Relu, bias=bias)
            h = hout

        if debug_stage == "mlp":
            nc.sync.dma_start(dbg_ext.ap()[:, :, :], h[:])
            return

        # ---------- last layer: natural-layout output ----------
        for m in range(BP // P):  # 16 batch chunks of 128
            pss = [mpsum.tile([P, NCHUNK], f32, name="mps")
                   for _ in range(QT)]
            for k in range(KT):
                for q in range(QT):
                    nc.tensor.matmul(pss[q][:], h[:, k, m * P:(m + 1) * P],
                                     wl_tiles[k // 2][:, k % 2, q * NQ:(q + 1) * NQ],
                                     start=(k == 0), stop=(k == KT - 1))
            r0 = m * P
            for q in range(QT):
                stg = stpool.tile([P, NQ], f32, name="stg")
                nc.vector.tensor_add(stg[:], pss[q][:],
                                     blb_sb[:, q * NQ:(q + 1) * NQ])
                nc.sync.dma_start(
                    out_ext.ap()[r0:r0 + P, q * NQ:(q + 1) * NQ], stg[:])


def prep_inputs(x1, x2, Ws, bs, W_last, b_last):
    """Host-side layout prep shared by all cores (weights) + per-core shards."""
    wt = np.ascontiguousarray(
        Ws.reshape(N_LAYERS, MT, P, KT // KG, KG, P)
        .transpose(0, 1, 3, 5, 4, 2)).astype(NP_BF16)
    wlt = np.ascontiguousarray(
        W_last.reshape(DOUT, KT // 2, 2, P).transpose(1, 3, 2, 0)).astype(NP_BF16)
    bst = np.ascontiguousarray(
        bs.reshape(N_LAYERS, MT, P).transpose(0, 2, 1))
    blb = np.ascontiguousarray(
        np.broadcast_to(b_last, (P, DOUT))).astype(NP_BF16)
    xz = np.ascontiguousarray(
        np.stack([x1, x2], axis=1)).astype(NP_BF16)
    shared = {"wt": wt, "wlt": wlt, "bst": bst, "blb": blb}
    in_maps = []
    for c in range(N_CORES):
        sl = slice(c * BC, (c + 1) * BC)
        m = {"xz": np.ascontiguousarray(xz[sl])}
        m.update(shared)
        in_maps.append(m)
    return in_maps


_compiled_nc = None


def kernel(x1, x2, Ws, bs, W_last, b_last):
    global _compiled_nc
    x1 = np.asarray(x1, dtype=np.float32)
    x2 = np.asarray(x2, dtype=np.float32)
    Ws = np.asarray(Ws, dtype=np.float32)
    bs = np.asarray(bs, dtype=np.float32)
    W_last = np.asarray(W_last, dtype=np.float32)
    b_last = np.asarray(b_last, dtype=np.float32)

    if _compiled_nc is None:
        _compiled_nc = build_graph()
    in_maps = prep_inputs(x1, x2, Ws, bs, W_last, b_last)
    res = run_bass_kernel_spmd(_compiled_nc, in_maps,
                               core_ids=list(range(N_CORES)))
    out = np.concatenate([res.results[c]["out"] for c in range(N_CORES)],
                         axis=0)
    return out.astype(np.float32)
